# revision 45
# baseline (speedup 1.0000x reference)
"""MLA attention kernel for TRN2, SPMD over 8 NeuronCores.

Sharding: core c = 4*b + g  (b = batch 0..1, g = head-group 0..3, 4 heads each).
Each core computes, for its batch b and head-group g:
    qT = (Wq_g*scale*A)^T x^T + bq_g*scale*A   [256, 2048]   (bf16; A=128/ln2 folded
                                                for fast-exp; undone at exp)
    latT = Wl^T x^T + bl                        [256, 2048]
    kT = Wk_g^T latT                            [256, 2048]   (bk dropped: softmax shift-invariant)
    v  = latT^T Wv_g                            [2048, 256]   (bv folded into host const)
    per head h: sT = kT_h^T qT_h ; pT = exp(sT/A)
                exp split: head h0 on ScalarE (table exp, scale=1/A), head h1
                on VectorE via a Schraudolph fast-exp (bf16 bits = sT + B,
                int16 write cast); the bias error cancels in softmax
    l = 1^T pT ; oT = v_h^T pT ; aT = oT * reciprocal_approx_fast(l)
    partial = A Wo_g                            [2048, 1024]  (bf16 out)
Host sums the 4 partials per batch (f32) and adds (bv @ Wo + bo).

Schedule: lag-2 software pipeline (PV + L-sum chunk g-2 ride QK filler g so
the ~1.2us VectorE fast-exp latency sits a full key-tile group off the
critical chain).  PSUM: scores + wo pairs + qt/proj transients share a
3x[128,2,512] ring (tag s); the PV accumulator is a single [128,512] bank
(tag ot) handed off pair-to-pair via an early psum->sbuf bf16 copy (which
also feeds an all-SBUF at-normalization mul on DVE); L + bc psums serialize
through one misc bank (tag m).  ScalarE: s0 table-exps + half the drain
copies; VectorE: s1 Schraudolph exps + recip + other half; engines get at
most one extra drain per filler slot.  norm(ic-1) is staged g0..g6 of
pair(ic,0); wo(ic-1) pairs late-p0..mid-p1; qt(ic+1) late-p1.  x chunk 0 +
weights DMA first at full fan-out; x1-3/wo chain behind x0 via GpSimd dummy
WAW copies so the first projection starts ~most of the DMA head earlier.
"""
import contextlib
import ctypes
import os
import sys
import types

if "/opt/trn_rl_repo" not in sys.path:
    sys.path.insert(0, "/opt/trn_rl_repo")

import numpy as np
import ml_dtypes

NPBF16 = ml_dtypes.bfloat16
SCALE = 64 ** -0.5
EXP_A = 128.0 / float(np.log(2.0))        # score pre-scale folded into Wq
EXP_INV_A = float(np.log(2.0)) / 128.0    # undo on ScalarE exp
SCHRAUD_B = 127.0 * 128.0 - 0.0579 * 128.0  # mean-centered Schraudolph bias
_STATE = {}


# ---------------------------------------------------------------- ntff shim
def _install_ntff_shim():
    """Provide antenv.axon_hooks so run_bass_kernel_spmd(trace=True) works."""
    if "antenv.axon_hooks" in sys.modules:
        return
    try:
        import antenv
    except ImportError:
        return

    so_path = "/opt/axon/libaxon_pjrt.so"

    def _hook_factory():
        try:
            lib = ctypes.CDLL(so_path)
        except OSError:
            return None
        if not hasattr(lib, "axon_start_nrt_profile"):
            return None
        lib.axon_start_nrt_profile.argtypes = [ctypes.POINTER(ctypes.c_int64), ctypes.c_size_t]
        lib.axon_start_nrt_profile.restype = ctypes.c_int64
        lib.axon_stop_nrt_profile.argtypes = [ctypes.c_char_p]
        lib.axon_stop_nrt_profile.restype = ctypes.c_int64

        @contextlib.contextmanager
        def _hook(output_dir, device_ids):
            import jax

            jax.devices()
            if device_ids:
                ids = (ctypes.c_int64 * len(device_ids))(*device_ids)
                rc = lib.axon_start_nrt_profile(ids, len(device_ids))
            else:
                rc = lib.axon_start_nrt_profile(None, 0)
            if rc != 0:
                raise RuntimeError(f"axon_start_nrt_profile rc={rc}")
            try:
                yield
            finally:
                n = lib.axon_stop_nrt_profile(str(output_dir).encode())
                print(f"profile: {n} file(s) written to {output_dir}", file=sys.stderr)

        return _hook

    import antenv

    mod = types.ModuleType("antenv.axon_hooks")
    _state = {"hook": _hook_factory()}
    mod.set_axon_ntff_profile_hook = lambda h: _state.__setitem__("hook", h)
    mod.get_axon_ntff_profile_hook = lambda: _state["hook"]
    sys.modules["antenv.axon_hooks"] = mod
    antenv.axon_hooks = mod


# ---------------------------------------------------------------- bass build
def _build_nc(debug_dump=False):
    import concourse.bass as bass  # noqa: F401
    import concourse.tile as tile
    from concourse import bacc, mybir

    BF16 = mybir.dt.bfloat16
    F32 = mybir.dt.float32
    I16 = mybir.dt.int16
    EXP = mybir.ActivationFunctionType.Exp
    IDENT = mybir.ActivationFunctionType.Identity
    ADD = mybir.AluOpType.add

    nc = bacc.Bacc(None, target_bir_lowering=False, debug=False)

    xT = nc.dram_tensor("xT", [128, 4, 8, 512], BF16, kind="ExternalInput")
    wq = nc.dram_tensor("wq", [128, 8, 256], BF16, kind="ExternalInput")
    bq = nc.dram_tensor("bq", [128, 2], F32, kind="ExternalInput")
    wl = nc.dram_tensor("wl", [128, 8, 256], BF16, kind="ExternalInput")
    bl = nc.dram_tensor("bl", [128, 2], F32, kind="ExternalInput")
    wk = nc.dram_tensor("wk", [128, 2, 256], BF16, kind="ExternalInput")
    wv = nc.dram_tensor("wv", [128, 2, 256], BF16, kind="ExternalInput")
    wo = nc.dram_tensor("wo", [128, 2, 1024], BF16, kind="ExternalInput")
    out = nc.dram_tensor("out", [2048, 1024], BF16, kind="ExternalOutput")

    GROUPS = [(2 * i, 2 * i + 2) for i in range(8)]
    GLEN = 2

    with nc.allow_low_precision("bf16 intermediates by design"), tile.TileContext(nc) as tc:
        with (
            tc.tile_pool(name="wpool", bufs=1) as wpool,
            tc.tile_pool(name="xpool", bufs=1) as xpool,
            tc.tile_pool(name="proj", bufs=1) as proj,
            tc.tile_pool(name="ptp", bufs=48) as ptp,
            tc.tile_pool(name="atp", bufs=8) as atp,
            tc.tile_pool(name="obp", bufs=4) as obp,
            tc.tile_pool(name="rpool", bufs=4) as rpool,
            tc.tile_pool(name="ps", bufs=2, space="PSUM") as ps,
        ):
            # ---------------- constants + inputs
            x_n = [xpool.tile([128, 8, 512], BF16, name=f"x_{n}") for n in range(4)]
            wq_sb = wpool.tile([128, 8, 256], BF16)
            wl_sb = wpool.tile([128, 8, 256], BF16)
            wk_sb = wpool.tile([128, 2, 256], BF16)
            wv_sb = wpool.tile([128, 2, 256], BF16)
            wo_sb = wpool.tile([128, 2, 1024], BF16)
            bq_sb = wpool.tile([128, 2], F32)
            bl_sb = wpool.tile([128, 2], F32)
            ones_sb = wpool.tile([128, 1], BF16)
            ones_k1 = wpool.tile([128, 64], BF16)

            # priority DMA: x chunk 0 + weights first at full fan-out; x1-3
            # and wo chained behind x0 via dummy WAW deps so the critical
            # first-projection inputs get the whole aggregate DMA bandwidth
            # interleave x0/wq chunk triggers: SP issues DMA triggers serially
            # at ~660ns each, so the qt(0)-critical inputs get the earliest
            # trigger slots with just enough stream parallelism
            nc.sync.dma_start(out=wq_sb[:, 0:4, :], in_=wq[:, 0:4, :])
            nc.sync.dma_start(out=x_n[0][:, 0:2, :], in_=xT[:, 0, 0:2, :])
            nc.sync.dma_start(out=x_n[0][:, 2:4, :], in_=xT[:, 0, 2:4, :])
            nc.sync.dma_start(out=wq_sb[:, 4:8, :], in_=wq[:, 4:8, :])
            nc.sync.dma_start(out=x_n[0][:, 4:6, :], in_=xT[:, 0, 4:6, :])
            nc.sync.dma_start(out=x_n[0][:, 6:8, :], in_=xT[:, 0, 6:8, :])
            nc.sync.dma_start(out=bq_sb[:], in_=bq[:])
            nc.sync.dma_start(out=wl_sb[:, 0:4, :], in_=wl[:, 0:4, :])
            nc.sync.dma_start(out=wl_sb[:, 4:8, :], in_=wl[:, 4:8, :])
            nc.sync.dma_start(out=bl_sb[:], in_=bl[:])
            nc.sync.dma_start(out=wk_sb[:], in_=wk[:])
            nc.sync.dma_start(out=wv_sb[:], in_=wv[:])
            for n in range(1, 4):
                nc.gpsimd.tensor_copy(
                    out=x_n[n][0:1, 0:1, 0:1], in_=x_n[n - 1][0:1, 0:1, 0:1]
                )
                nc.gpsimd.tensor_copy(
                    out=x_n[n][0:1, 4:5, 0:1], in_=x_n[n - 1][0:1, 0:1, 0:1]
                )
                nc.sync.dma_start(out=x_n[n][:, 0:4, :], in_=xT[:, n, 0:4, :])
                nc.sync.dma_start(out=x_n[n][:, 4:8, :], in_=xT[:, n, 4:8, :])
            nc.gpsimd.tensor_copy(out=wo_sb[0:1, 0:1, 0:1], in_=x_n[1][0:1, 0:1, 0:1])
            nc.sync.dma_start(out=wo_sb[:], in_=wo[:])
            nc.vector.memset(ones_sb[:], 1.0)
            nc.vector.memset(ones_k1[:], 1.0)

            latT_n = [proj.tile([128, 2, 512], BF16, name=f"latT_{i}") for i in range(4)]
            qT_n = [proj.tile([128, 2, 512], BF16, name=f"qT_{i}") for i in range(4)]
            kT_n = [proj.tile([128, 2, 512], BF16, name=f"kT_{i}") for i in range(4)]
            v_sb = proj.tile([128, 16, 256], BF16)

            # PSUM: tag "s" 3x[128,2,512] (scores + wo pairs + ic0/qt/bc
            # transients), tag "ot" 1x[128,512] (PV accum, handed off via an
            # early psum->sbuf copy), tag "m" 1x[128,512] (L + bc/qt psums)
            def ot_ps(name):
                return ps.tile([128, 512], F32, tag="ot", name=name, bufs=1)

            def misc_ps(name):
                return ps.tile([128, 512], F32, tag="m", name=name, bufs=1)

            def sring_ps(name):
                return ps.tile([128, 512], F32, tag="s", name=name, bufs=3)

            def sring_ps2(name):
                return ps.tile([128, 2, 512], F32, tag="s", name=name, bufs=3)

            # HAM warm-up: dummy matmuls while input DMA is in flight
            warm_sb = wpool.tile([128, 512], BF16)
            nc.vector.memset(warm_sb[:], 0.25)
            warm_ps = sring_ps("warm_ps")
            for i in range(12):
                nc.tensor.matmul(
                    warm_ps[:], warm_sb[:, 0:128], warm_sb[:],
                    start=(i == 0), stop=(i == 11),
                )

            # ---------------- projection emitters
            def emit_lat_n(n):
                for m in range(2):
                    acc = sring_ps(f"lat_ps_{m}_{n}")
                    for k in range(8):
                        nc.tensor.matmul(
                            acc[:],
                            wl_sb[:, k, 128 * m : 128 * m + 128],
                            x_n[n][:, k, :],
                            start=(k == 0),
                            stop=(k == 7),
                        )
                    nc.scalar.activation(
                        latT_n[n][:, m, :], acc[:], IDENT, bias=bl_sb[:, m : m + 1]
                    )

            def emit_kt_n(n):
                for m in range(2):
                    acc = sring_ps(f"kt_ps_{m}_{n}")
                    for k in range(2):
                        nc.tensor.matmul(
                            acc[:],
                            wk_sb[:, k, 128 * m : 128 * m + 128],
                            latT_n[n][:, k, :],
                            start=(k == 0),
                            stop=(k == 1),
                        )
                    nc.vector.tensor_copy(out=kT_n[n][:, m, :], in_=acc[:])

            def emit_v(ts):
                for t in ts:
                    acc = sring_ps(f"v_ps_{t}")
                    for k in range(2):
                        nc.tensor.matmul(
                            acc[:, 0:256],
                            latT_n[t // 4][:, k, 128 * (t % 4) : 128 * (t % 4) + 128],
                            wv_sb[:, k, :],
                            start=(k == 0),
                            stop=(k == 1),
                        )
                    if t % 2 == 0:
                        nc.scalar.copy(out=v_sb[:, t, :], in_=acc[:, 0:256])
                    else:
                        nc.vector.tensor_copy(out=v_sb[:, t, :], in_=acc[:, 0:256])

            # qt split: emit_qt_mm(ic, m) then emit_qt_copy(ic, m)
            qt_accs = {}

            def emit_qt_mm(ic, m):
                qt_accs[(ic, m)] = sring_ps(f"q_ps_{m}_{ic}")
                acc = qt_accs[(ic, m)]
                for k in range(8):
                    nc.tensor.matmul(
                        acc[:],
                        wq_sb[:, k, 128 * m : 128 * m + 128],
                        x_n[ic][:, k, :],
                        start=(k == 0),
                        stop=(k == 7),
                    )

            def emit_qt_copy(ic, m, engine="scalar"):
                acc = qt_accs.pop((ic, m))
                if engine == "scalar":
                    nc.scalar.activation(
                        qT_n[ic][:, m, :], acc[:], IDENT, bias=bq_sb[:, m : m + 1]
                    )
                else:
                    nc.vector.tensor_scalar(
                        out=qT_n[ic][:, m, :], in0=acc[:],
                        scalar1=bq_sb[:, m : m + 1], scalar2=None, op0=ADD,
                    )

            def emit_qt_full(ic):
                for m in range(2):
                    emit_qt_mm(ic, m)
                    emit_qt_copy(ic, m)

            # ---------------- attention emitters
            pds = {}

            def emit_pv_chunk(pd, gi):
                if pd["ot"] is None:
                    pd["ot"] = ot_ps(f"ot_{pd['ic']}_{pd['p']}")
                ot0 = pd["ot"]
                h0, h1 = 2 * pd["p"], 2 * pd["p"] + 1
                t0, t1 = GROUPS[gi]
                pt0, pt1 = pd["pts"][gi]
                for t in range(t0, t1):
                    tt = t - t0
                    nc.tensor.matmul(
                        ot0[0:64, :], v_sb[:, t, 64 * h0 : 64 * h0 + 64], pt0[:, tt, :],
                        start=(t == 0), stop=(t == 15), skip_group_check=True,
                    )
                    nc.tensor.matmul(
                        ot0[64:128, :], v_sb[:, t, 64 * h1 : 64 * h1 + 64], pt1[:, tt, :],
                        start=(t == 0), stop=(t == 15), skip_group_check=True,
                    )

            Ls = {}

            def emit_sums_chunk(ic, pair_data, gi):
                if gi == 0:
                    L = misc_ps(f"L_{ic}")
                    nc.vector.memset(L[:], 1.0)
                    Ls[ic] = L
                L = Ls[ic]
                t0, t1 = GROUPS[gi]
                for t in range(t0, t1):
                    tt = t - t0
                    for p in range(2):
                        pt0, pt1 = pair_data[p]["pts"][gi]
                        for hh, pt in ((2 * p, pt0), (2 * p + 1, pt1)):
                            nc.tensor.matmul(
                                L[32 * hh : 32 * hh + 1, :],
                                ones_sb[:],
                                pt[:, tt, :],
                                start=(t == 0),
                                stop=(t == 15),
                                tile_position=(0, 32 * hh),
                                skip_group_check=True,
                            )

            # early ot psum -> sbuf copy: frees the single ot bank and feeds
            # the all-SBUF bf16 at-mul later
            ot_sbs = {}

            def emit_ot_copy(ic, p, engine="scalar"):
                ot = pds[(ic, p)]["ot"]
                ot_sb = atp.tile([128, 512], BF16, tag="otsb", name=f"otsb_{ic}_{p}")
                if engine == "scalar":
                    nc.scalar.copy(out=ot_sb[:], in_=ot[:])
                else:
                    nc.vector.tensor_copy(out=ot_sb[:], in_=ot[:])
                ot_sbs[(ic, p)] = ot_sb

            bcbs = {}
            bcs = {}
            recipbs = {}

            def emit_recip(ic):
                # recip + bf16 cast (DVE); bc matmuls staged per-p through the
                # single misc psum slot; at-mul = ot_sb * bc all-SBUF on DVE
                L = Ls.pop(ic)
                recip = rpool.tile([128, 512], F32, tag="recip", name=f"recip_{ic}", bufs=1)
                nc.vector.reciprocal_approx_fast(out=recip[:], in_=L[:])
                recipb = rpool.tile([128, 512], BF16, tag="recipb", name=f"recipb_{ic}", bufs=1)
                nc.scalar.copy(out=recipb[:], in_=recip[:])
                recipbs[ic] = recipb

            def emit_bc_mm(ic, p):
                recipb = recipbs[ic]
                if p == 1:
                    del recipbs[ic]
                bc_ps = misc_ps(f"bcp_{ic}_{p}")
                for j, hh in enumerate((2 * p, 2 * p + 1)):
                    rb = 32 * hh
                    nc.tensor.matmul(
                        bc_ps[64 * j : 64 * j + 64, :],
                        ones_k1[rb : rb + 1, :],
                        recipb[rb : rb + 1, :],
                        start=True,
                        stop=True,
                        tile_position=(rb, 64 * j),
                        skip_group_check=True,
                    )
                bcbs[(ic, p)] = bc_ps

            def emit_bc_copy(ic, p, engine="scalar"):
                bc_ps = bcbs.pop((ic, p))
                bc = rpool.tile([128, 512], BF16, tag="bc", name=f"bc_{ic}_{p}", bufs=2)
                if engine == "scalar":
                    nc.scalar.copy(out=bc[:], in_=bc_ps[:])
                else:
                    nc.vector.tensor_copy(out=bc[:], in_=bc_ps[:])
                bcs[(ic, p)] = bc

            ats = {}

            def emit_at_mul(ic, p):
                bc = bcs.pop((ic, p))
                at = atp.tile([128, 512], BF16, tag="at", name=f"at_{ic}_{p}")
                ot_sb = ot_sbs.pop((ic, p))
                nc.vector.tensor_mul(out=at[:], in0=ot_sb[:], in1=bc[:])
                ats[(ic, p)] = at

            # wo as 4-MM pairs into one [128,2,512] s-ring tile; 512-col copy
            # halves split across engines one filler later
            wo_ps_refs = {}
            ob_tiles = {}

            def emit_wo_mms(ic, u):
                wo_ps = sring_ps2(f"wo_{ic}_{u}")
                for n2 in range(2):
                    for p in range(2):
                        nc.tensor.matmul(
                            wo_ps[:, n2, :],
                            ats[(ic, p)][:, 128 * u : 128 * u + 128],
                            wo_sb[:, p, 512 * n2 : 512 * n2 + 512],
                            start=(p == 0),
                            stop=(p == 1),
                        )
                wo_ps_refs[(ic, u)] = wo_ps

            def emit_wo_copy_half(ic, u, n2, engine="scalar"):
                wo_ps = wo_ps_refs[(ic, u)]
                if n2 == 1:
                    del wo_ps_refs[(ic, u)]
                if (ic, u) not in ob_tiles:
                    ob_tiles[(ic, u)] = obp.tile(
                        [128, 2, 512], BF16, tag="ob", name=f"ob_{ic}_{u}"
                    )
                ob = ob_tiles[(ic, u)]
                if engine == "scalar":
                    nc.scalar.copy(out=ob[:, n2, :], in_=wo_ps[:, n2, :])
                else:
                    nc.vector.tensor_copy(out=ob[:, n2, :], in_=wo_ps[:, n2, :])

            def emit_ob_dma(ic, u):
                ob = ob_tiles.pop((ic, u))
                r0 = 512 * ic + 128 * u
                nc.sync.dma_start(out=out[r0 : r0 + 128, :], in_=ob[:])

            # ---------------- pair runner: fully table-driven fillers
            def run_pair(ic, p, post):
                pds[(ic, p)] = {"pts": [], "ot": None, "ic": ic, "p": p}
                me = pds[(ic, p)]
                qTc = qT_n[ic]

                def filler(g):
                    for f in post.get(g, ()):
                        f()

                for gi, (t0, t1) in enumerate(GROUPS):
                    s0 = sring_ps2(f"s0_{ic}_{p}_{gi}")
                    s1 = sring_ps2(f"s1_{ic}_{p}_{gi}")
                    for t in range(t0, t1):
                        tt = t - t0
                        kTc = kT_n[t // 4]
                        ksl = slice(128 * (t % 4), 128 * (t % 4) + 128)
                        nc.tensor.matmul(
                            s0[:, tt, :], kTc[0:64, p, ksl], qTc[0:64, p, :],
                            start=True, stop=True,
                        )
                        nc.tensor.matmul(
                            s1[:, tt, :], kTc[64:128, p, ksl], qTc[64:128, p, :],
                            start=True, stop=True,
                        )
                    pt0 = ptp.tile([128, GLEN, 512], BF16, tag="pt", name=f"pt0_{ic}_{p}_{gi}")
                    pt1 = ptp.tile([128, GLEN, 512], BF16, tag="pt", name=f"pt1_{ic}_{p}_{gi}")
                    nc.scalar.activation(pt0[:], s0[:], EXP, scale=EXP_INV_A)
                    nc.vector.tensor_scalar(
                        out=pt1[:].bitcast(I16), in0=s1[:],
                        scalar1=SCHRAUD_B, scalar2=None, op0=ADD,
                    )
                    me["pts"].append((pt0, pt1))
                    filler(gi)
                return me

            # ---------------- schedule
            emit_qt_full(0)
            emit_lat_n(0)
            emit_kt_n(0)
            emit_v(range(0, 4))

            # Schedule (lag-2): PV+sums chunk g-2 ride filler g, so the DVE
            # fast-exp latency sits a full gi off the critical chain.  norm
            # chain of ic-1 staged through pair(ic,0); wo pairs late-p0 to
            # mid-p1; qt(ic+1) late-p1.  One extra op per engine per filler.
            def pv(ic, p, c):
                return lambda: emit_pv_chunk(pds[(ic, p)], c)

            def sums(ic, c):
                return lambda: emit_sums_chunk(ic, [pds[(ic, 0)], pds[(ic, 1)]], c)

            def p0_post(ic):
                jc = ic - 1
                return {
                    0: [pv(jc, 1, 6), sums(jc, 6)],
                    1: [pv(jc, 1, 7), sums(jc, 7),
                        lambda: emit_ot_copy(jc, 1, "scalar"),
                        lambda: emit_recip(jc)],
                    2: [pv(ic, 0, 0), lambda: emit_bc_mm(jc, 0)],
                    3: [pv(ic, 0, 1), lambda: emit_bc_copy(jc, 0, "scalar")],
                    4: [pv(ic, 0, 2), lambda: emit_at_mul(jc, 0),
                        lambda: emit_bc_mm(jc, 1)],
                    5: [pv(ic, 0, 3), lambda: emit_bc_copy(jc, 1, "vector")],
                    6: [pv(ic, 0, 4), lambda: emit_at_mul(jc, 1),
                        lambda: emit_wo_mms(jc, 0)],
                    7: [pv(ic, 0, 5),
                        lambda: emit_wo_copy_half(jc, 0, 0, "scalar"),
                        lambda: emit_wo_copy_half(jc, 0, 1, "vector"),
                        lambda: emit_wo_mms(jc, 1)],
                }

            def p1_post(ic):
                jc = ic - 1
                nxt = ic + 1
                post = {
                    0: [pv(ic, 0, 6),
                        lambda: emit_wo_copy_half(jc, 1, 0, "scalar"),
                        lambda: emit_wo_copy_half(jc, 1, 1, "vector"),
                        lambda: emit_ob_dma(jc, 0)],
                    1: [pv(ic, 0, 7), lambda: emit_ot_copy(ic, 0, "scalar"),
                        lambda: emit_ob_dma(jc, 1)],
                    2: [pv(ic, 1, 0), sums(ic, 0), lambda: emit_wo_mms(jc, 2)],
                    3: [pv(ic, 1, 1), sums(ic, 1),
                        lambda: emit_wo_copy_half(jc, 2, 0, "scalar"),
                        lambda: emit_wo_copy_half(jc, 2, 1, "vector")],
                    4: [pv(ic, 1, 2), sums(ic, 2), lambda: emit_wo_mms(jc, 3),
                        lambda: emit_ob_dma(jc, 2)],
                    5: [pv(ic, 1, 3), sums(ic, 3),
                        lambda: emit_wo_copy_half(jc, 3, 0, "scalar"),
                        lambda: emit_wo_copy_half(jc, 3, 1, "vector")],
                    6: [pv(ic, 1, 4), sums(ic, 4), lambda: emit_ob_dma(jc, 3)],
                    7: [pv(ic, 1, 5), sums(ic, 5)],
                }
                if nxt <= 3:
                    post[5].append(lambda: emit_qt_mm(nxt, 0))
                    post[6] += [lambda: emit_qt_copy(nxt, 0, "scalar"),
                                lambda: emit_qt_mm(nxt, 1)]
                    post[7].append(lambda: emit_qt_copy(nxt, 1, "scalar"))
                return post

            # ic = 0: projection work rides in pair(0,0); qt(1) in pair(0,1)
            post00 = {
                1: [lambda: (emit_lat_n(1), emit_kt_n(1))],
                2: [pv(0, 0, 0), lambda: emit_v(range(4, 8))],
                3: [pv(0, 0, 1), lambda: (emit_lat_n(2), emit_kt_n(2))],
                4: [pv(0, 0, 2), lambda: emit_v(range(8, 12))],
                5: [pv(0, 0, 3), lambda: (emit_lat_n(3), emit_kt_n(3))],
                6: [pv(0, 0, 4), lambda: emit_v(range(12, 16))],
                7: [pv(0, 0, 5)],
            }
            post01 = {
                0: [pv(0, 0, 6)],
                1: [pv(0, 0, 7), lambda: emit_ot_copy(0, 0, "scalar")],
                2: [pv(0, 1, 0), sums(0, 0)],
                3: [pv(0, 1, 1), sums(0, 1)],
                4: [pv(0, 1, 2), sums(0, 2), lambda: emit_qt_mm(1, 0)],
                5: [pv(0, 1, 3), sums(0, 3),
                    lambda: emit_qt_copy(1, 0, "scalar"), lambda: emit_qt_mm(1, 1)],
                6: [pv(0, 1, 4), sums(0, 4), lambda: emit_qt_copy(1, 1, "scalar")],
                7: [pv(0, 1, 5), sums(0, 5)],
            }

            run_pair(0, 0, post00)
            run_pair(0, 1, post01)
            for ic in range(1, 4):
                run_pair(ic, 0, p0_post(ic))
                run_pair(ic, 1, p1_post(ic))

            # ---------------- tail: finish pair(3,1) + norm(3) + wo(3)
            emit_pv_chunk(pds[(3, 1)], 6)
            emit_sums_chunk(3, [pds[(3, 0)], pds[(3, 1)]], 6)
            emit_pv_chunk(pds[(3, 1)], 7)
            emit_sums_chunk(3, [pds[(3, 0)], pds[(3, 1)]], 7)
            emit_ot_copy(3, 1, "scalar")
            emit_recip(3)
            emit_bc_mm(3, 0)
            emit_bc_copy(3, 0, "scalar")
            emit_at_mul(3, 0)
            emit_bc_mm(3, 1)
            emit_bc_copy(3, 1, "vector")
            emit_at_mul(3, 1)
            # front-load the tail wo matmuls (3 s-ring slots) so the PE
            # streams them back-to-back while the copies pipeline behind
            emit_wo_mms(3, 0)
            emit_wo_mms(3, 1)
            emit_wo_mms(3, 2)
            emit_wo_copy_half(3, 0, 0, "scalar")
            emit_wo_copy_half(3, 0, 1, "vector")
            emit_wo_mms(3, 3)
            emit_ob_dma(3, 0)
            emit_wo_copy_half(3, 1, 0, "scalar")
            emit_wo_copy_half(3, 1, 1, "vector")
            emit_ob_dma(3, 1)

            def emit_ob_dma_split(u):
                # final DMAs: two parallel partition-half streams (2KB rows
                # preserved) so the exposed last transfer halves in time
                ob = ob_tiles.pop((3, u))
                r0 = 512 * 3 + 128 * u
                nc.sync.dma_start(out=out[r0 : r0 + 64, :], in_=ob[0:64, :, :])
                nc.sync.dma_start(out=out[r0 + 64 : r0 + 128, :], in_=ob[64:128, :, :])

            emit_wo_copy_half(3, 2, 0, "scalar")
            emit_wo_copy_half(3, 2, 1, "vector")
            emit_ob_dma_split(2)
            emit_wo_copy_half(3, 3, 0, "scalar")
            emit_wo_copy_half(3, 3, 1, "vector")
            emit_ob_dma_split(3)

    nc.compile()
    return nc


def _get_nc():
    if "nc" not in _STATE:
        _STATE["nc"] = _build_nc()
    return _STATE["nc"]


# ---------------------------------------------------------------- host side
def _pack_k(a, kchunks):
    """[K, N] f32/bf16 -> [128, kchunks, N] bf16 (K = 128*kchunks)."""
    K, N = a.shape
    return np.ascontiguousarray(
        np.asarray(a, np.float32).reshape(kchunks, 128, N).transpose(1, 0, 2)
    ).astype(NPBF16)


def _pack_x(xb):
    """x[b] [2048, 1024] -> xT packed [128, 4, 8, 512] (n-major, 8KB lines)."""
    xT = np.asarray(xb, np.float32).T  # [1024, 2048]
    return np.ascontiguousarray(
        xT.reshape(8, 128, 4, 512).transpose(1, 2, 0, 3)
    ).astype(NPBF16)


def kernel(x, Wq, bq, Wl, bl, Wk, bk, Wv, bv, Wo, bo):
    x = np.asarray(x, np.float32)
    Wq = np.asarray(Wq, np.float32)
    bq = np.asarray(bq, np.float32)
    Wl = np.asarray(Wl, np.float32)
    bl = np.asarray(bl, np.float32)
    Wk = np.asarray(Wk, np.float32)
    Wv = np.asarray(Wv, np.float32)
    bv = np.asarray(bv, np.float32)
    Wo = np.asarray(Wo, np.float32)
    bo = np.asarray(bo, np.float32)

    from concourse.bass_utils import run_bass_kernel_spmd

    trace = os.environ.get("KERNEL_TRACE", "0") == "1"
    if trace:
        _install_ntff_shim()

    qscale = SCALE * EXP_A
    wl_p = _pack_k(Wl, 8)
    bl_p = np.ascontiguousarray(bl.reshape(2, 128).T).astype(np.float32)
    x_p = [_pack_x(x[b]) for b in range(2)]
    in_maps = []
    for c in range(8):
        b, g = divmod(c, 4)
        sl = slice(256 * g, 256 * g + 256)
        in_maps.append(
            {
                "xT": x_p[b],
                "wq": _pack_k(Wq[:, sl] * qscale, 8),
                "bq": np.ascontiguousarray((bq[sl] * qscale).reshape(2, 128).T).astype(np.float32),
                "wl": wl_p,
                "bl": bl_p,
                "wk": _pack_k(Wk[:, sl], 2),
                "wv": _pack_k(Wv[:, sl], 2),
                "wo": _pack_k(Wo[sl, :], 2),
            }
        )

    nc = _get_nc()
    res = run_bass_kernel_spmd(nc, in_maps, core_ids=list(range(8)), trace=trace)
    if trace and res.exec_time_ns is not None:
        print(f"HW exec time: {res.exec_time_ns} ns")
        _STATE["exec_time_ns"] = res.exec_time_ns

    parts = [np.asarray(res.results[c]["out"], np.float32) for c in range(8)]
    const = (bv @ Wo + bo).astype(np.float32)
    out = np.empty((2, 2048, 1024), np.float32)
    for b in range(2):
        out[b] = parts[4 * b] + parts[4 * b + 1] + parts[4 * b + 2] + parts[4 * b + 3] + const
    return out


# revision 46
# speedup vs baseline: 1.0106x; 1.0106x over previous
"""MLA attention kernel for TRN2, SPMD over 8 NeuronCores.

Sharding: core c = 4*b + g  (b = batch 0..1, g = head-group 0..3, 4 heads each).
Each core computes, for its batch b and head-group g:
    qT = (Wq_g*scale*A)^T x^T + bq_g*scale*A   [256, 2048]   (bf16; A=128/ln2 folded
                                                for fast-exp; undone at exp)
    latT = Wl^T x^T + bl                        [256, 2048]
    kT = Wk_g^T latT                            [256, 2048]   (bk dropped: softmax shift-invariant)
    v  = latT^T Wv_g                            [2048, 256]   (bv folded into host const)
    per head h: sT = kT_h^T qT_h ; pT = exp(sT/A)
                exp split: head h0 on ScalarE (table exp, scale=1/A), head h1
                on VectorE via a Schraudolph fast-exp (bf16 bits = sT + B,
                int16 write cast); the bias error cancels in softmax
    l = 1^T pT ; oT = v_h^T pT ; aT = oT * reciprocal_approx_fast(l)
    partial = A Wo_g                            [2048, 1024]  (bf16 out)
Host sums the 4 partials per batch (f32) and adds (bv @ Wo + bo).

Schedule: lag-2 software pipeline (PV + L-sum chunk g-2 ride QK filler g so
the ~1.2us VectorE fast-exp latency sits a full key-tile group off the
critical chain).  PSUM: scores + wo pairs + qt/proj transients share a
3x[128,2,512] ring (tag s); the PV accumulator is a single [128,512] bank
(tag ot) handed off pair-to-pair via an early psum->sbuf bf16 copy (which
also feeds an all-SBUF at-normalization mul on DVE); L + bc psums serialize
through one misc bank (tag m).  ScalarE: s0 table-exps + half the drain
copies; VectorE: s1 Schraudolph exps + recip + other half; engines get at
most one extra drain per filler slot.  norm(ic-1) is staged g0..g6 of
pair(ic,0); wo(ic-1) pairs late-p0..mid-p1; qt(ic+1) late-p1.  x chunk 0 +
weights DMA first at full fan-out; x1-3/wo chain behind x0 via GpSimd dummy
WAW copies so the first projection starts ~most of the DMA head earlier.
"""
import contextlib
import ctypes
import os
import sys
import types

if "/opt/trn_rl_repo" not in sys.path:
    sys.path.insert(0, "/opt/trn_rl_repo")

import numpy as np
import ml_dtypes

NPBF16 = ml_dtypes.bfloat16
SCALE = 64 ** -0.5
EXP_A = 128.0 / float(np.log(2.0))        # score pre-scale folded into Wq
EXP_INV_A = float(np.log(2.0)) / 128.0    # undo on ScalarE exp
SCHRAUD_B = 127.0 * 128.0 - 0.0579 * 128.0  # mean-centered Schraudolph bias
_STATE = {}


# ---------------------------------------------------------------- ntff shim
def _install_ntff_shim():
    """Provide antenv.axon_hooks so run_bass_kernel_spmd(trace=True) works."""
    if "antenv.axon_hooks" in sys.modules:
        return
    try:
        import antenv
    except ImportError:
        return

    so_path = "/opt/axon/libaxon_pjrt.so"

    def _hook_factory():
        try:
            lib = ctypes.CDLL(so_path)
        except OSError:
            return None
        if not hasattr(lib, "axon_start_nrt_profile"):
            return None
        lib.axon_start_nrt_profile.argtypes = [ctypes.POINTER(ctypes.c_int64), ctypes.c_size_t]
        lib.axon_start_nrt_profile.restype = ctypes.c_int64
        lib.axon_stop_nrt_profile.argtypes = [ctypes.c_char_p]
        lib.axon_stop_nrt_profile.restype = ctypes.c_int64

        @contextlib.contextmanager
        def _hook(output_dir, device_ids):
            import jax

            jax.devices()
            if device_ids:
                ids = (ctypes.c_int64 * len(device_ids))(*device_ids)
                rc = lib.axon_start_nrt_profile(ids, len(device_ids))
            else:
                rc = lib.axon_start_nrt_profile(None, 0)
            if rc != 0:
                raise RuntimeError(f"axon_start_nrt_profile rc={rc}")
            try:
                yield
            finally:
                n = lib.axon_stop_nrt_profile(str(output_dir).encode())
                print(f"profile: {n} file(s) written to {output_dir}", file=sys.stderr)

        return _hook

    import antenv

    mod = types.ModuleType("antenv.axon_hooks")
    _state = {"hook": _hook_factory()}
    mod.set_axon_ntff_profile_hook = lambda h: _state.__setitem__("hook", h)
    mod.get_axon_ntff_profile_hook = lambda: _state["hook"]
    sys.modules["antenv.axon_hooks"] = mod
    antenv.axon_hooks = mod


# ---------------------------------------------------------------- bass build
def _build_nc(debug_dump=False):
    import concourse.bass as bass  # noqa: F401
    import concourse.tile as tile
    from concourse import bacc, mybir

    BF16 = mybir.dt.bfloat16
    F32 = mybir.dt.float32
    I16 = mybir.dt.int16
    EXP = mybir.ActivationFunctionType.Exp
    IDENT = mybir.ActivationFunctionType.Identity
    ADD = mybir.AluOpType.add

    nc = bacc.Bacc(None, target_bir_lowering=False, debug=False)

    xT = nc.dram_tensor("xT", [128, 4, 8, 512], BF16, kind="ExternalInput")
    wq = nc.dram_tensor("wq", [128, 8, 256], BF16, kind="ExternalInput")
    bq = nc.dram_tensor("bq", [128, 2], F32, kind="ExternalInput")
    wl = nc.dram_tensor("wl", [128, 8, 256], BF16, kind="ExternalInput")
    bl = nc.dram_tensor("bl", [128, 2], F32, kind="ExternalInput")
    wk = nc.dram_tensor("wk", [128, 2, 256], BF16, kind="ExternalInput")
    wv = nc.dram_tensor("wv", [128, 2, 256], BF16, kind="ExternalInput")
    wo = nc.dram_tensor("wo", [128, 2, 1024], BF16, kind="ExternalInput")
    out = nc.dram_tensor("out", [2048, 1024], BF16, kind="ExternalOutput")

    GROUPS = [(2 * i, 2 * i + 2) for i in range(8)]
    GLEN = 2

    with nc.allow_low_precision("bf16 intermediates by design"), tile.TileContext(nc) as tc:
        with (
            tc.tile_pool(name="wpool", bufs=1) as wpool,
            tc.tile_pool(name="xpool", bufs=1) as xpool,
            tc.tile_pool(name="proj", bufs=1) as proj,
            tc.tile_pool(name="ptp", bufs=48) as ptp,
            tc.tile_pool(name="atp", bufs=8) as atp,
            tc.tile_pool(name="obp", bufs=4) as obp,
            tc.tile_pool(name="rpool", bufs=4) as rpool,
            tc.tile_pool(name="ps", bufs=2, space="PSUM") as ps,
        ):
            # ---------------- constants + inputs
            x_n = [xpool.tile([128, 8, 512], BF16, name=f"x_{n}") for n in range(4)]
            wq_sb = wpool.tile([128, 8, 256], BF16)
            wl_sb = wpool.tile([128, 8, 256], BF16)
            wk_sb = wpool.tile([128, 2, 256], BF16)
            wv_sb = wpool.tile([128, 2, 256], BF16)
            wo_sb = wpool.tile([128, 2, 1024], BF16)
            bq_sb = wpool.tile([128, 2], F32)
            bl_sb = wpool.tile([128, 2], F32)
            ones_sb = wpool.tile([128, 1], BF16)
            ones_k1 = wpool.tile([128, 64], BF16)

            # priority DMA: x chunk 0 + weights first at full fan-out; x1-3
            # and wo chained behind x0 via dummy WAW deps so the critical
            # first-projection inputs get the whole aggregate DMA bandwidth
            # interleave x0/wq chunk triggers: SP issues DMA triggers serially
            # at ~660ns each, so the qt(0)-critical inputs get the earliest
            # trigger slots with just enough stream parallelism
            nc.sync.dma_start(out=wq_sb[:, 0:4, :], in_=wq[:, 0:4, :])
            nc.sync.dma_start(out=x_n[0][:, 0:2, :], in_=xT[:, 0, 0:2, :])
            nc.sync.dma_start(out=x_n[0][:, 2:4, :], in_=xT[:, 0, 2:4, :])
            nc.sync.dma_start(out=wq_sb[:, 4:8, :], in_=wq[:, 4:8, :])
            nc.sync.dma_start(out=x_n[0][:, 4:6, :], in_=xT[:, 0, 4:6, :])
            nc.sync.dma_start(out=x_n[0][:, 6:8, :], in_=xT[:, 0, 6:8, :])
            nc.sync.dma_start(out=bq_sb[:], in_=bq[:])
            nc.sync.dma_start(out=wl_sb[:, 0:4, :], in_=wl[:, 0:4, :])
            nc.sync.dma_start(out=wl_sb[:, 4:8, :], in_=wl[:, 4:8, :])
            nc.sync.dma_start(out=bl_sb[:], in_=bl[:])
            nc.sync.dma_start(out=wk_sb[:], in_=wk[:])
            nc.sync.dma_start(out=wv_sb[:], in_=wv[:])
            for n in range(1, 4):
                nc.gpsimd.tensor_copy(
                    out=x_n[n][0:1, 0:1, 0:1], in_=x_n[n - 1][0:1, 0:1, 0:1]
                )
                nc.gpsimd.tensor_copy(
                    out=x_n[n][0:1, 4:5, 0:1], in_=x_n[n - 1][0:1, 0:1, 0:1]
                )
                nc.sync.dma_start(out=x_n[n][:, 0:4, :], in_=xT[:, n, 0:4, :])
                nc.sync.dma_start(out=x_n[n][:, 4:8, :], in_=xT[:, n, 4:8, :])
            nc.gpsimd.tensor_copy(out=wo_sb[0:1, 0:1, 0:1], in_=x_n[1][0:1, 0:1, 0:1])
            nc.sync.dma_start(out=wo_sb[:], in_=wo[:])
            nc.vector.memset(ones_sb[:], 1.0)
            nc.vector.memset(ones_k1[:], 1.0)

            latT_n = [proj.tile([128, 2, 512], BF16, name=f"latT_{i}") for i in range(4)]
            qT_n = [proj.tile([128, 2, 512], BF16, name=f"qT_{i}") for i in range(4)]
            kT_n = [proj.tile([128, 2, 512], BF16, name=f"kT_{i}") for i in range(4)]
            v_sb = proj.tile([128, 16, 256], BF16)

            # PSUM: tag "s" 3x[128,2,512] (scores + wo pairs + ic0/qt/bc
            # transients), tag "ot" 1x[128,512] (PV accum, handed off via an
            # early psum->sbuf copy), tag "m" 1x[128,512] (L + bc/qt psums)
            def ot_ps(name):
                return ps.tile([128, 512], F32, tag="ot", name=name, bufs=1)

            def misc_ps(name):
                return ps.tile([128, 512], F32, tag="m", name=name, bufs=1)

            def sring_ps(name):
                return ps.tile([128, 512], F32, tag="s", name=name, bufs=3)

            def sring_ps2(name):
                return ps.tile([128, 2, 512], F32, tag="s", name=name, bufs=3)

            # HAM warm-up: dummy matmuls while input DMA is in flight
            warm_sb = wpool.tile([128, 512], BF16)
            nc.vector.memset(warm_sb[:], 0.25)
            warm_ps = sring_ps("warm_ps")
            for i in range(12):
                nc.tensor.matmul(
                    warm_ps[:], warm_sb[:, 0:128], warm_sb[:],
                    start=(i == 0), stop=(i == 11),
                )

            # ---------------- projection emitters
            def emit_lat_n(n):
                for m in range(2):
                    acc = sring_ps(f"lat_ps_{m}_{n}")
                    for k in range(8):
                        nc.tensor.matmul(
                            acc[:],
                            wl_sb[:, k, 128 * m : 128 * m + 128],
                            x_n[n][:, k, :],
                            start=(k == 0),
                            stop=(k == 7),
                        )
                    nc.scalar.activation(
                        latT_n[n][:, m, :], acc[:], IDENT, bias=bl_sb[:, m : m + 1]
                    )

            def emit_kt_n(n):
                for m in range(2):
                    acc = sring_ps(f"kt_ps_{m}_{n}")
                    for k in range(2):
                        nc.tensor.matmul(
                            acc[:],
                            wk_sb[:, k, 128 * m : 128 * m + 128],
                            latT_n[n][:, k, :],
                            start=(k == 0),
                            stop=(k == 1),
                        )
                    nc.vector.tensor_copy(out=kT_n[n][:, m, :], in_=acc[:])

            def emit_v(ts):
                for t in ts:
                    acc = sring_ps(f"v_ps_{t}")
                    for k in range(2):
                        nc.tensor.matmul(
                            acc[:, 0:256],
                            latT_n[t // 4][:, k, 128 * (t % 4) : 128 * (t % 4) + 128],
                            wv_sb[:, k, :],
                            start=(k == 0),
                            stop=(k == 1),
                        )
                    if t % 2 == 0:
                        nc.scalar.copy(out=v_sb[:, t, :], in_=acc[:, 0:256])
                    else:
                        nc.vector.tensor_copy(out=v_sb[:, t, :], in_=acc[:, 0:256])

            # qt split: emit_qt_mm(ic, m) then emit_qt_copy(ic, m)
            qt_accs = {}

            def emit_qt_mm(ic, m):
                qt_accs[(ic, m)] = sring_ps(f"q_ps_{m}_{ic}")
                acc = qt_accs[(ic, m)]
                for k in range(8):
                    nc.tensor.matmul(
                        acc[:],
                        wq_sb[:, k, 128 * m : 128 * m + 128],
                        x_n[ic][:, k, :],
                        start=(k == 0),
                        stop=(k == 7),
                    )

            def emit_qt_copy(ic, m, engine="scalar"):
                acc = qt_accs.pop((ic, m))
                if engine == "scalar":
                    nc.scalar.activation(
                        qT_n[ic][:, m, :], acc[:], IDENT, bias=bq_sb[:, m : m + 1]
                    )
                else:
                    nc.vector.tensor_scalar(
                        out=qT_n[ic][:, m, :], in0=acc[:],
                        scalar1=bq_sb[:, m : m + 1], scalar2=None, op0=ADD,
                    )

            def emit_qt_full(ic):
                for m in range(2):
                    emit_qt_mm(ic, m)
                    emit_qt_copy(ic, m)

            # ---------------- attention emitters
            pds = {}

            def emit_pv_chunk(pd, gi):
                if pd["ot"] is None:
                    pd["ot"] = ot_ps(f"ot_{pd['ic']}_{pd['p']}")
                ot0 = pd["ot"]
                h0, h1 = 2 * pd["p"], 2 * pd["p"] + 1
                t0, t1 = GROUPS[gi]
                pt0, pt1 = pd["pts"][gi]
                for t in range(t0, t1):
                    tt = t - t0
                    nc.tensor.matmul(
                        ot0[0:64, :], v_sb[:, t, 64 * h0 : 64 * h0 + 64], pt0[:, tt, :],
                        start=(t == 0), stop=(t == 15), skip_group_check=True,
                    )
                    nc.tensor.matmul(
                        ot0[64:128, :], v_sb[:, t, 64 * h1 : 64 * h1 + 64], pt1[:, tt, :],
                        start=(t == 0), stop=(t == 15), skip_group_check=True,
                    )

            Ls = {}

            def emit_sums_chunk(ic, pair_data, gi):
                if gi == 0:
                    L = misc_ps(f"L_{ic}")
                    nc.vector.memset(L[:], 1.0)
                    Ls[ic] = L
                L = Ls[ic]
                t0, t1 = GROUPS[gi]
                for t in range(t0, t1):
                    tt = t - t0
                    for p in range(2):
                        pt0, pt1 = pair_data[p]["pts"][gi]
                        for hh, pt in ((2 * p, pt0), (2 * p + 1, pt1)):
                            nc.tensor.matmul(
                                L[32 * hh : 32 * hh + 1, :],
                                ones_sb[:],
                                pt[:, tt, :],
                                start=(t == 0),
                                stop=(t == 15),
                                tile_position=(0, 32 * hh),
                                skip_group_check=True,
                            )

            # early ot psum -> sbuf copy: frees the single ot bank and feeds
            # the all-SBUF bf16 at-mul later
            ot_sbs = {}

            def emit_ot_copy(ic, p, engine="scalar"):
                ot = pds[(ic, p)]["ot"]
                ot_sb = atp.tile([128, 512], BF16, tag="otsb", name=f"otsb_{ic}_{p}")
                if engine == "scalar":
                    nc.scalar.copy(out=ot_sb[:], in_=ot[:])
                else:
                    nc.vector.tensor_copy(out=ot_sb[:], in_=ot[:])
                ot_sbs[(ic, p)] = ot_sb

            bcbs = {}
            bcs = {}
            recipbs = {}

            def emit_recip(ic):
                # recip + bf16 cast (DVE); bc matmuls staged per-p through the
                # single misc psum slot; at-mul = ot_sb * bc all-SBUF on DVE
                L = Ls.pop(ic)
                recip = rpool.tile([128, 512], F32, tag="recip", name=f"recip_{ic}", bufs=1)
                nc.vector.reciprocal_approx_fast(out=recip[:], in_=L[:])
                recipb = rpool.tile([128, 512], BF16, tag="recipb", name=f"recipb_{ic}", bufs=1)
                nc.scalar.copy(out=recipb[:], in_=recip[:])
                recipbs[ic] = recipb

            def emit_bc_mm(ic, p):
                recipb = recipbs[ic]
                if p == 1:
                    del recipbs[ic]
                bc_ps = misc_ps(f"bcp_{ic}_{p}")
                for j, hh in enumerate((2 * p, 2 * p + 1)):
                    rb = 32 * hh
                    nc.tensor.matmul(
                        bc_ps[64 * j : 64 * j + 64, :],
                        ones_k1[rb : rb + 1, :],
                        recipb[rb : rb + 1, :],
                        start=True,
                        stop=True,
                        tile_position=(rb, 64 * j),
                        skip_group_check=True,
                    )
                bcbs[(ic, p)] = bc_ps

            def emit_bc_copy(ic, p, engine="scalar"):
                bc_ps = bcbs.pop((ic, p))
                bc = rpool.tile([128, 512], BF16, tag="bc", name=f"bc_{ic}_{p}", bufs=2)
                if engine == "scalar":
                    nc.scalar.copy(out=bc[:], in_=bc_ps[:])
                else:
                    nc.vector.tensor_copy(out=bc[:], in_=bc_ps[:])
                bcs[(ic, p)] = bc

            ats = {}

            def emit_at_mul(ic, p):
                bc = bcs.pop((ic, p))
                at = atp.tile([128, 512], BF16, tag="at", name=f"at_{ic}_{p}")
                ot_sb = ot_sbs.pop((ic, p))
                nc.vector.tensor_mul(out=at[:], in0=ot_sb[:], in1=bc[:])
                ats[(ic, p)] = at

            # wo as 4-MM pairs into one [128,2,512] s-ring tile; 512-col copy
            # halves split across engines one filler later
            wo_ps_refs = {}
            ob_tiles = {}

            def emit_wo_mms(ic, u):
                wo_ps = sring_ps2(f"wo_{ic}_{u}")
                for n2 in range(2):
                    for p in range(2):
                        nc.tensor.matmul(
                            wo_ps[:, n2, :],
                            ats[(ic, p)][:, 128 * u : 128 * u + 128],
                            wo_sb[:, p, 512 * n2 : 512 * n2 + 512],
                            start=(p == 0),
                            stop=(p == 1),
                        )
                wo_ps_refs[(ic, u)] = wo_ps

            def emit_wo_copy_half(ic, u, n2, engine="scalar"):
                wo_ps = wo_ps_refs[(ic, u)]
                if n2 == 1:
                    del wo_ps_refs[(ic, u)]
                if (ic, u) not in ob_tiles:
                    ob_tiles[(ic, u)] = obp.tile(
                        [128, 2, 512], BF16, tag="ob", name=f"ob_{ic}_{u}"
                    )
                ob = ob_tiles[(ic, u)]
                if engine == "scalar":
                    nc.scalar.copy(out=ob[:, n2, :], in_=wo_ps[:, n2, :])
                else:
                    nc.vector.tensor_copy(out=ob[:, n2, :], in_=wo_ps[:, n2, :])

            def emit_ob_dma(ic, u):
                ob = ob_tiles.pop((ic, u))
                r0 = 512 * ic + 128 * u
                nc.sync.dma_start(out=out[r0 : r0 + 128, :], in_=ob[:])

            # ---------------- pair runner: fully table-driven fillers
            def run_pair(ic, p, post):
                pds[(ic, p)] = {"pts": [], "ot": None, "ic": ic, "p": p}
                me = pds[(ic, p)]
                qTc = qT_n[ic]

                def filler(g):
                    for f in post.get(g, ()):
                        f()

                for gi, (t0, t1) in enumerate(GROUPS):
                    s0 = sring_ps2(f"s0_{ic}_{p}_{gi}")
                    s1 = sring_ps2(f"s1_{ic}_{p}_{gi}")
                    for t in range(t0, t1):
                        tt = t - t0
                        kTc = kT_n[t // 4]
                        ksl = slice(128 * (t % 4), 128 * (t % 4) + 128)
                        nc.tensor.matmul(
                            s0[:, tt, :], kTc[0:64, p, ksl], qTc[0:64, p, :],
                            start=True, stop=True,
                        )
                        nc.tensor.matmul(
                            s1[:, tt, :], kTc[64:128, p, ksl], qTc[64:128, p, :],
                            start=True, stop=True,
                        )
                    pt0 = ptp.tile([128, GLEN, 512], BF16, tag="pt", name=f"pt0_{ic}_{p}_{gi}")
                    pt1 = ptp.tile([128, GLEN, 512], BF16, tag="pt", name=f"pt1_{ic}_{p}_{gi}")
                    nc.scalar.activation(pt0[:], s0[:], EXP, scale=EXP_INV_A)
                    nc.vector.tensor_scalar(
                        out=pt1[:].bitcast(I16), in0=s1[:],
                        scalar1=SCHRAUD_B, scalar2=None, op0=ADD,
                    )
                    me["pts"].append((pt0, pt1))
                    filler(gi)
                return me

            # ---------------- schedule
            emit_qt_full(0)
            emit_lat_n(0)
            emit_kt_n(0)
            emit_v(range(0, 4))

            # Schedule (lag-2): PV+sums chunk g-2 ride filler g, so the DVE
            # fast-exp latency sits a full gi off the critical chain.  norm
            # chain of ic-1 staged through pair(ic,0); wo pairs late-p0 to
            # mid-p1; qt(ic+1) late-p1.  One extra op per engine per filler.
            def pv(ic, p, c):
                return lambda: emit_pv_chunk(pds[(ic, p)], c)

            def sums(ic, c):
                return lambda: emit_sums_chunk(ic, [pds[(ic, 0)], pds[(ic, 1)]], c)

            def p0_post(ic):
                jc = ic - 1
                return {
                    0: [pv(jc, 1, 6), sums(jc, 6)],
                    1: [pv(jc, 1, 7), sums(jc, 7),
                        lambda: emit_ot_copy(jc, 1, "scalar"),
                        lambda: emit_recip(jc)],
                    2: [pv(ic, 0, 0), lambda: emit_bc_mm(jc, 0)],
                    3: [pv(ic, 0, 1), lambda: emit_bc_copy(jc, 0, "scalar")],
                    4: [pv(ic, 0, 2), lambda: emit_at_mul(jc, 0),
                        lambda: emit_bc_mm(jc, 1)],
                    5: [pv(ic, 0, 3), lambda: emit_bc_copy(jc, 1, "vector")],
                    6: [pv(ic, 0, 4), lambda: emit_at_mul(jc, 1),
                        lambda: emit_wo_mms(jc, 0)],
                    7: [pv(ic, 0, 5),
                        lambda: emit_wo_copy_half(jc, 0, 0, "scalar"),
                        lambda: emit_wo_copy_half(jc, 0, 1, "vector"),
                        lambda: emit_wo_mms(jc, 1)],
                }

            def p1_post(ic):
                jc = ic - 1
                nxt = ic + 1
                post = {
                    0: [pv(ic, 0, 6),
                        lambda: emit_wo_copy_half(jc, 1, 0, "scalar"),
                        lambda: emit_wo_copy_half(jc, 1, 1, "vector"),
                        lambda: emit_ob_dma(jc, 0)],
                    1: [pv(ic, 0, 7), lambda: emit_ot_copy(ic, 0, "scalar"),
                        lambda: emit_ob_dma(jc, 1)],
                    2: [pv(ic, 1, 0), sums(ic, 0), lambda: emit_wo_mms(jc, 2)],
                    3: [pv(ic, 1, 1), sums(ic, 1),
                        lambda: emit_wo_copy_half(jc, 2, 0, "scalar"),
                        lambda: emit_wo_copy_half(jc, 2, 1, "vector")],
                    4: [pv(ic, 1, 2), sums(ic, 2), lambda: emit_wo_mms(jc, 3),
                        lambda: emit_ob_dma(jc, 2)],
                    5: [pv(ic, 1, 3), sums(ic, 3),
                        lambda: emit_wo_copy_half(jc, 3, 0, "scalar"),
                        lambda: emit_wo_copy_half(jc, 3, 1, "vector")],
                    6: [pv(ic, 1, 4), sums(ic, 4), lambda: emit_ob_dma(jc, 3)],
                    7: [pv(ic, 1, 5), sums(ic, 5)],
                }
                if nxt <= 3:
                    post[5].append(lambda: emit_qt_mm(nxt, 0))
                    post[6] += [lambda: emit_qt_copy(nxt, 0, "scalar"),
                                lambda: emit_qt_mm(nxt, 1)]
                    post[7].append(lambda: emit_qt_copy(nxt, 1, "scalar"))
                return post

            # ic = 0: projection work rides in pair(0,0); qt(1) in pair(0,1)
            post00 = {
                1: [lambda: (emit_lat_n(1), emit_kt_n(1))],
                2: [pv(0, 0, 0), lambda: emit_v(range(4, 8))],
                3: [pv(0, 0, 1), lambda: (emit_lat_n(2), emit_kt_n(2))],
                4: [pv(0, 0, 2), lambda: emit_v(range(8, 12))],
                5: [pv(0, 0, 3), lambda: (emit_lat_n(3), emit_kt_n(3))],
                6: [pv(0, 0, 4), lambda: emit_v(range(12, 16))],
                7: [pv(0, 0, 5)],
            }
            post01 = {
                0: [pv(0, 0, 6)],
                1: [pv(0, 0, 7), lambda: emit_ot_copy(0, 0, "scalar")],
                2: [pv(0, 1, 0), sums(0, 0)],
                3: [pv(0, 1, 1), sums(0, 1)],
                4: [pv(0, 1, 2), sums(0, 2), lambda: emit_qt_mm(1, 0)],
                5: [pv(0, 1, 3), sums(0, 3),
                    lambda: emit_qt_copy(1, 0, "scalar"), lambda: emit_qt_mm(1, 1)],
                6: [pv(0, 1, 4), sums(0, 4), lambda: emit_qt_copy(1, 1, "scalar")],
                7: [pv(0, 1, 5), sums(0, 5)],
            }

            run_pair(0, 0, post00)
            run_pair(0, 1, post01)
            for ic in range(1, 4):
                run_pair(ic, 0, p0_post(ic))
                run_pair(ic, 1, p1_post(ic))

            # ---------------- tail: finish pair(3,1) + norm(3) + wo(3)
            emit_pv_chunk(pds[(3, 1)], 6)
            emit_sums_chunk(3, [pds[(3, 0)], pds[(3, 1)]], 6)
            emit_pv_chunk(pds[(3, 1)], 7)
            emit_sums_chunk(3, [pds[(3, 0)], pds[(3, 1)]], 7)
            emit_ot_copy(3, 1, "scalar")
            emit_recip(3)
            emit_bc_mm(3, 0)
            emit_bc_copy(3, 0, "scalar")
            emit_at_mul(3, 0)
            emit_bc_mm(3, 1)
            emit_bc_copy(3, 1, "vector")
            emit_at_mul(3, 1)
            # front-load the tail wo matmuls (3 s-ring slots) so the PE
            # streams them back-to-back while the copies pipeline behind
            emit_wo_mms(3, 0)
            emit_wo_mms(3, 1)
            emit_wo_mms(3, 2)
            emit_wo_copy_half(3, 0, 0, "scalar")
            emit_wo_copy_half(3, 0, 1, "vector")
            emit_wo_mms(3, 3)
            emit_ob_dma(3, 0)
            emit_wo_copy_half(3, 1, 0, "scalar")
            emit_wo_copy_half(3, 1, 1, "vector")
            emit_ob_dma(3, 1)
            emit_wo_copy_half(3, 2, 0, "scalar")
            emit_wo_copy_half(3, 2, 1, "vector")
            emit_ob_dma(3, 2)
            emit_wo_copy_half(3, 3, 0, "scalar")
            emit_wo_copy_half(3, 3, 1, "vector")
            emit_ob_dma(3, 3)

    nc.compile()
    return nc


def _get_nc():
    if "nc" not in _STATE:
        _STATE["nc"] = _build_nc()
    return _STATE["nc"]


# ---------------------------------------------------------------- host side
def _pack_k(a, kchunks):
    """[K, N] f32/bf16 -> [128, kchunks, N] bf16 (K = 128*kchunks)."""
    K, N = a.shape
    return np.ascontiguousarray(
        np.asarray(a, np.float32).reshape(kchunks, 128, N).transpose(1, 0, 2)
    ).astype(NPBF16)


def _pack_x(xb):
    """x[b] [2048, 1024] -> xT packed [128, 4, 8, 512] (n-major, 8KB lines)."""
    xT = np.asarray(xb, np.float32).T  # [1024, 2048]
    return np.ascontiguousarray(
        xT.reshape(8, 128, 4, 512).transpose(1, 2, 0, 3)
    ).astype(NPBF16)


def kernel(x, Wq, bq, Wl, bl, Wk, bk, Wv, bv, Wo, bo):
    x = np.asarray(x, np.float32)
    Wq = np.asarray(Wq, np.float32)
    bq = np.asarray(bq, np.float32)
    Wl = np.asarray(Wl, np.float32)
    bl = np.asarray(bl, np.float32)
    Wk = np.asarray(Wk, np.float32)
    Wv = np.asarray(Wv, np.float32)
    bv = np.asarray(bv, np.float32)
    Wo = np.asarray(Wo, np.float32)
    bo = np.asarray(bo, np.float32)

    from concourse.bass_utils import run_bass_kernel_spmd

    trace = os.environ.get("KERNEL_TRACE", "0") == "1"
    if trace:
        _install_ntff_shim()

    qscale = SCALE * EXP_A
    wl_p = _pack_k(Wl, 8)
    bl_p = np.ascontiguousarray(bl.reshape(2, 128).T).astype(np.float32)
    x_p = [_pack_x(x[b]) for b in range(2)]
    in_maps = []
    for c in range(8):
        b, g = divmod(c, 4)
        sl = slice(256 * g, 256 * g + 256)
        in_maps.append(
            {
                "xT": x_p[b],
                "wq": _pack_k(Wq[:, sl] * qscale, 8),
                "bq": np.ascontiguousarray((bq[sl] * qscale).reshape(2, 128).T).astype(np.float32),
                "wl": wl_p,
                "bl": bl_p,
                "wk": _pack_k(Wk[:, sl], 2),
                "wv": _pack_k(Wv[:, sl], 2),
                "wo": _pack_k(Wo[sl, :], 2),
            }
        )

    nc = _get_nc()
    res = run_bass_kernel_spmd(nc, in_maps, core_ids=list(range(8)), trace=trace)
    if trace and res.exec_time_ns is not None:
        print(f"HW exec time: {res.exec_time_ns} ns")
        _STATE["exec_time_ns"] = res.exec_time_ns

    parts = [np.asarray(res.results[c]["out"], np.float32) for c in range(8)]
    const = (bv @ Wo + bo).astype(np.float32)
    out = np.empty((2, 2048, 1024), np.float32)
    for b in range(2):
        out[b] = parts[4 * b] + parts[4 * b + 1] + parts[4 * b + 2] + parts[4 * b + 3] + const
    return out
